# revision 29
# baseline (speedup 1.0000x reference)
"""Fused multi-head attention with dropout for Trainium2 (Bass/Tile), 8-core SPMD.

Problem: out = dropout(softmax(Q @ K^T * scale)) @ V
  Q/K/V: [64, 2048, 64] fp32, dropout_mask: [64, 2048, 2048] fp32, p = 0.5.

Sharding: the 64 batch*heads are split across 8 NeuronCores (8 heads/core),
no cross-device communication.

Per-head device algorithm (head-local, S = 2048, D = 64):
  Scores are computed TRANSPOSED, S^T[k, q] = K @ Q^T, so softmax rows (over
  k) land on the partition axis and the PV product needs no on-chip transpose:
  O^T[d, q] = sum_k V[k, d] * P[k, q] accumulates in PSUM.

  Engine balance (the baseline was PE-bound at ~327us/core because the
  softmax denominator sum_k exp(s) was a ones-matmul per k-chunk - 1/3 of
  all PE cycles - with Vector near-saturated and GpSimd idle):
   - Act: exp only ([128,1024] fp32 PSUM tiles -> bf16 SBUF), nothing else;
          at 1.2G rows/s + ~185ns/instr this is the ~266us pipeline pacer.
   - PE:  QK (fp32r) + PV (bf16) + denominator ones-matmuls for PE_CHUNKS
          + accumulator folds at end of slice (end placement matters: a
          fold mid-slice stalls the in-order PE queue on the DVE chain).
   - DVE: dropout mask-mult as all-bf16 tensor_tensor (the 2x_1p DVE mode
          needs every operand 2-byte; HW-measured 682ns/[128,1024] tile vs
          3.7us for any u8-mixed op), two bf16 denominator chunk-sum
          accumulators, reciprocal + final output multiply, both deferred
          into the next slice so they stay off the critical path.
   - GpSimd: nothing (a dependent gpsimd op costs ~10us pipeline latency on
          HW, and even its software-DGE DMAs measure ~20us slower).
  Chunk-major [128,1024] score tiles keep one stationary load per two
  matmuls (a 512-wide variant that halved the accumulator banks measured
  +24us from doubled stationary reloads, unmodeled in CoreSim).
  Masks ship as bf16 {0,1} with a 5-chunk-lead DMA cursor; head tensors
  prefetch ~1.5 slices early so their 7us of DMA never starves the mask
  stream. The 1/(1-p)=2 dropout rescale is folded into the 0.5-valued ones
  weights: out = oacc / (0.5 * sum_k exp).
"""

import numpy as np
from contextlib import ExitStack

import concourse.bass as bass
import concourse.bacc as bacc
import concourse.tile as tile
import concourse.mybir as mybir
from concourse.bass_utils import run_bass_kernel_spmd

N_CORES = 8
B, S, D = 64, 2048, 64
HPC = B // N_CORES  # heads per core
KP = 128            # k-chunk size (PSUM partition dim)
QL = 1024           # q-slice width (one [128,1024] PSUM score tile = 2 banks)
NQ = 512            # matmul moving free-dim tile (one fp32 PSUM bank)
DROP_P = 0.5
N_KC = S // KP      # 16 k-chunks
MK_LEAD = 6         # mask DMA prefetch lead, in chunks

# Chunks whose denominator ones-matmul runs directly on PE (rest are summed
# in bf16 on DVE, two accumulators, folded by PE at end of slice).
PE_CHUNKS = tuple(range(5))


def build_program(
    n_heads=HPC,
    seq=S,
    d=D,
    scale=1.0,
    reps=1,
    pe_chunks=PE_CHUNKS,
    mk_lead=MK_LEAD,
):
    f32 = mybir.dt.float32
    bf16 = mybir.dt.bfloat16
    # float32r: same fp32 bytes, PE streams 1 col/cycle (vs 4 for fp32) at
    # ~tf32 precision (HW-probed maxabs 5.8e-3 on N(0,64) scores).
    fmm = mybir.dt.float32r
    n_kc = seq // KP
    n_qh = seq // QL
    n_j = QL // NQ
    pe_set = set(c for c in pe_chunks if c < n_kc)
    dve_accs = [c for c in range(n_kc) if c not in pe_set]
    acc_of = {}
    for i, c in enumerate(dve_accs):
        acc_of[c] = 0 if i < (len(dve_accs) + 1) // 2 else 1

    nc = bacc.Bacc("TRN2", target_bir_lowering=False, debug=False)
    qt_d = nc.dram_tensor("qt", [n_heads, d, seq], fmm, kind="ExternalInput").ap()
    kt_d = nc.dram_tensor("kt", [n_heads, d, seq], fmm, kind="ExternalInput").ap()
    vp_d = nc.dram_tensor("vp", [n_heads, KP, n_kc * d], bf16, kind="ExternalInput").ap()
    mt_d = nc.dram_tensor("mt", [n_heads, seq, seq], bf16, kind="ExternalInput").ap()
    ot_d = nc.dram_tensor("ot", [n_heads, d, seq], bf16, kind="ExternalOutput").ap()

    # Software-pipelined emission over a flat list of (head, q-slice) blocks:
    # per chunk c the program order is [dma mask(cursor)] [exp(c)] [QK(next)]
    # [mask-mult(c)] [PV(c)] [denom(c)], so each engine's in-order stream
    # never waits on the current chunk's cross-engine chain.
    blocks = [(h, qh) for h in range(n_heads) for qh in range(n_qh)] * reps

    with tile.TileContext(nc) as tc:
        with ExitStack() as ctx:
            const = ctx.enter_context(tc.tile_pool(name="const", bufs=1))
            qkv = ctx.enter_context(tc.tile_pool(name="qkv", bufs=2))
            mpool = ctx.enter_context(tc.tile_pool(name="mask", bufs=14))
            ppool = ctx.enter_context(tc.tile_pool(name="p", bufs=8))
            dpool = ctx.enter_context(tc.tile_pool(name="pd", bufs=5))
            apool = ctx.enter_context(tc.tile_pool(name="acc", bufs=4))
            opool = ctx.enter_context(tc.tile_pool(name="o", bufs=3))
            # PSUM budget (8 banks): st 2x2 + oacc 2 + oden 2.
            pst = ctx.enter_context(
                tc.tile_pool(name="pst", bufs=2, space=bass.MemorySpace.PSUM)
            )
            pacc = ctx.enter_context(
                tc.tile_pool(name="pacc", bufs=1, space=bass.MemorySpace.PSUM)
            )
            pden = ctx.enter_context(
                tc.tile_pool(name="pden", bufs=1, space=bass.MemorySpace.PSUM)
            )

            # d identical 0.5-columns: the denominator matmul then emits
            # 0.5*sum_k already replicated across the d output partitions,
            # and the 0.5 folds the dropout 1/(1-p)=2 rescale into the
            # final reciprocal.
            ones = const.tile([KP, d], bf16)
            nc.vector.memset(ones[:], 0.5)

            head_tiles: dict = {}

            def load_head(h):
                qt_sb = qkv.tile([d, seq], fmm, tag="qt")
                nc.sync.dma_start(qt_sb[:], qt_d[h])
                kt_sb = qkv.tile([d, seq], fmm, tag="kt")
                nc.sync.dma_start(kt_sb[:], kt_d[h])
                v_sb = qkv.tile([KP, n_kc * d], bf16, tag="v")
                nc.sync.dma_start(v_sb[:], vp_d[h])
                head_tiles[h] = (qt_sb, kt_sb, v_sb)

            mk_tiles: dict = {}
            st_tiles: dict = {}

            def dma_mk(b, c):
                h, qh = blocks[b]
                q0 = qh * QL
                t = mpool.tile([KP, QL], bf16, tag="mk")
                nc.sync.dma_start(t[:], mt_d[h, c * KP : (c + 1) * KP, q0 : q0 + QL])
                mk_tiles[(b, c)] = t

            def qk(b, c):
                h, qh = blocks[b]
                q0 = qh * QL
                qt_sb, kt_sb, _ = head_tiles[h]
                t = pst.tile([KP, QL], f32, tag="st")
                for j in range(n_j):
                    nc.tensor.matmul(
                        t[:, j * NQ : (j + 1) * NQ],
                        kt_sb[:, c * KP : (c + 1) * KP],
                        qt_sb[:, q0 + j * NQ : q0 + (j + 1) * NQ],
                        start=True,
                        stop=True,
                    )
                st_tiles[(b, c)] = t

            mk_sched = [(bb, cc) for bb in range(len(blocks)) for cc in range(n_kc)]
            mk_cursor = [0]

            def advance_mk(n):
                for _ in range(n):
                    if mk_cursor[0] < len(mk_sched):
                        dma_mk(*mk_sched[mk_cursor[0]])
                        mk_cursor[0] += 1

            load_head(0)
            advance_mk(mk_lead)
            qk(0, 0)

            pe_sorted = sorted(pe_set)
            half = (len(dve_accs) + 1) // 2
            n_dsrc = (
                len(pe_sorted) + (1 if half else 0) + (1 if len(dve_accs) - half else 0)
            )

            pending = [None, None]  # deferred out-stage compute / dma

            for b, (h, qh) in enumerate(blocks):
                _, _, v_sb = head_tiles[h]
                oacc = pacc.tile([d, QL], f32, tag="oacc")
                oden = pden.tile([d, QL], f32, tag="oden")
                accs = [None, None]
                pend = [None, None]  # first p0 of an accumulator pair
                dsrc = [0]

                def oden_fold(src, oden=oden, dsrc=dsrc):
                    for j in range(n_j):
                        nc.tensor.matmul(
                            oden[:, j * NQ : (j + 1) * NQ],
                            ones,
                            src[:, j * NQ : (j + 1) * NQ],
                            start=dsrc[0] == 0,
                            stop=dsrc[0] == n_dsrc - 1,
                        )
                    dsrc[0] += 1

                for c in range(n_kc):
                    # prefetch the next head's tensors halfway through this
                    # head's FIRST slice (~17us lead over first use)
                    if (
                        c == n_kc // 2
                        and qh == 0
                        and b + 2 < len(blocks)
                        and blocks[b + 2][0] != h
                    ):
                        load_head(blocks[b + 2][0])
                    advance_mk(1)

                    st = st_tiles.pop((b, c))
                    p0 = ppool.tile([KP, QL], bf16, tag="p0")
                    nc.scalar.activation(
                        p0[:], st[:], mybir.ActivationFunctionType.Exp, scale=scale
                    )
                    nxt = (b, c + 1) if c + 1 < n_kc else (b + 1, 0)
                    if nxt[0] < len(blocks):
                        qk(*nxt)
                    mk = mk_tiles.pop((b, c))
                    pd = dpool.tile([KP, QL], bf16, tag="pd")
                    nc.vector.tensor_tensor(pd[:], mk[:], p0[:], mybir.AluOpType.mult)
                    if c == 0 and pending[0] is not None:
                        pending[1] = pending[0]()
                        pending[0] = None
                    elif c == 1 and pending[1] is not None:
                        pending[1]()
                        pending[1] = None
                    first, last = c == 0, c == n_kc - 1
                    for j in range(n_j):
                        nc.tensor.matmul(
                            oacc[:, j * NQ : (j + 1) * NQ],
                            v_sb[:, c * d : (c + 1) * d],
                            pd[:, j * NQ : (j + 1) * NQ],
                            start=first,
                            stop=last,
                        )
                    # denominator contribution of this chunk
                    if c in pe_set:
                        oden_fold(p0)
                    else:
                        ai = acc_of[c]
                        if accs[ai] is None and pend[ai] is None:
                            pend[ai] = p0
                        elif accs[ai] is None:
                            t = apool.tile([KP, QL], bf16, tag="acc")
                            nc.vector.tensor_tensor(
                                t[:], pend[ai][:], p0[:], mybir.AluOpType.add
                            )
                            accs[ai] = t
                            pend[ai] = None
                        else:
                            nc.vector.tensor_tensor(
                                accs[ai][:], accs[ai][:], p0[:], mybir.AluOpType.add
                            )

                # fold the two bf16 accumulators into the PSUM denominator
                for acc in accs:
                    if acc is not None:
                        oden_fold(acc)
                for pp in pend:
                    if pp is not None:
                        oden_fold(pp)

                # out = oacc * (1 / (0.5 * sum_k exp)); the compute defers to
                # the next slice's first mask-mult and the store one chunk
                # further, keeping both off the in-order critical paths.
                def make_out(h=h, qh=qh, oacc=oacc, oden=oden):
                    def emit():
                        q0 = qh * QL
                        rb = opool.tile([d, QL], f32, tag="rb")
                        nc.vector.reciprocal_approx_fast(rb[:], oden[:])
                        out_sb = opool.tile([d, QL], bf16, tag="out")
                        nc.vector.tensor_tensor(
                            out_sb[:], oacc[:], rb[:], mybir.AluOpType.mult
                        )

                        def emit_dma():
                            nc.sync.dma_start(ot_d[h, :, q0 : q0 + QL], out_sb[:])

                        return emit_dma

                    return emit

                pending[0] = make_out()
            if pending[0] is not None:
                pending[1] = pending[0]()
            if pending[1] is not None:
                pending[1]()

    nc.compile()
    return nc


_CACHE: dict = {}


def _get_program(scale: float):
    key = float(scale)
    if key not in _CACHE:
        _CACHE[key] = build_program(scale=key)
    return _CACHE[key]


def make_in_maps(query, key, value, dropout_mask, **_ignored):
    """Shard + relayout the full inputs into the 8 per-core input maps."""
    import ml_dtypes

    query = np.asarray(query, dtype=np.float32)
    key = np.asarray(key, dtype=np.float32)
    value = np.asarray(value, dtype=np.float32)
    dropout_mask = np.asarray(dropout_mask, dtype=np.float32)
    in_maps = []
    for cid in range(N_CORES):
        sl = slice(cid * HPC, (cid + 1) * HPC)
        qt = np.ascontiguousarray(query[sl].transpose(0, 2, 1))
        kt = np.ascontiguousarray(key[sl].transpose(0, 2, 1))
        vp = np.ascontiguousarray(
            value[sl].reshape(HPC, S // KP, KP, D).transpose(0, 2, 1, 3)
        ).reshape(HPC, KP, (S // KP) * D).astype(ml_dtypes.bfloat16)
        mt = (dropout_mask[sl].transpose(0, 2, 1) >= DROP_P).astype(
            ml_dtypes.bfloat16
        )  # [h, k, q] keep-mask
        in_maps.append({"qt": qt, "kt": kt, "vp": vp, "mt": mt})
    return in_maps


def run(query, key, value, scale_factor, dropout_mask, trace=False, **trace_kwargs):
    scale = float(np.asarray(scale_factor).reshape(()))
    nc = _get_program(scale)
    in_maps = make_in_maps(query, key, value, dropout_mask)
    res = run_bass_kernel_spmd(
        nc, in_maps, core_ids=list(range(N_CORES)), trace=trace, **trace_kwargs
    )
    outs = [
        np.asarray(res.results[c]["ot"]).astype(np.float32).transpose(0, 2, 1)
        for c in range(N_CORES)
    ]
    full = np.ascontiguousarray(np.concatenate(outs, axis=0), dtype=np.float32)
    return full, res


def kernel(query, key, value, scale_factor, dropout_mask):
    out, _ = run(query, key, value, scale_factor, dropout_mask, trace=False)
    return out


# revision 30
# speedup vs baseline: 1.2318x; 1.2318x over previous
"""Fused multi-head attention with dropout for Trainium2 (Bass/Tile), 8-core SPMD.

Problem: out = dropout(softmax(Q @ K^T * scale)) @ V
  Q/K/V: [64, 2048, 64] fp32, dropout_mask: [64, 2048, 2048] fp32, p = 0.5.

Sharding: the 64 batch*heads are split across 8 NeuronCores (8 heads/core),
no cross-device communication.

Per-head device algorithm (head-local, S = 2048, D = 64):
  Scores are computed TRANSPOSED, S^T[k, q] = K @ Q^T, so softmax rows (over
  k) land on the partition axis and the PV product needs no on-chip transpose:
  O^T[d, q] = sum_k V[k, d] * P[k, q] accumulates in PSUM.

  Engine balance (the baseline was PE-bound at ~327us/core because the
  softmax denominator sum_k exp(s) was a ones-matmul per k-chunk - 1/3 of
  all PE cycles - with Vector near-saturated and GpSimd idle):
   - Act: exp only ([128,1024] fp32 PSUM tiles -> bf16 SBUF), nothing else;
          at 1.2G rows/s + ~185ns/instr this is the ~266us pipeline pacer.
   - PE:  QK (fp32r) + PV (bf16) + denominator ones-matmuls for PE_CHUNKS
          + accumulator folds at end of slice (end placement matters: a
          fold mid-slice stalls the in-order PE queue on the DVE chain).
   - DVE: dropout mask-mult as all-bf16 tensor_tensor (the 2x_1p DVE mode
          needs every operand 2-byte; HW-measured 682ns/[128,1024] tile vs
          3.7us for any u8-mixed op), two bf16 denominator chunk-sum
          accumulators, reciprocal + final output multiply, both deferred
          into the next slice so they stay off the critical path.
   - GpSimd: nothing (a dependent gpsimd op costs ~10us pipeline latency on
          HW, and even its software-DGE DMAs measure ~20us slower).
  Chunk-major [128,1024] score tiles keep one stationary load per two
  matmuls (a 512-wide variant that halved the accumulator banks measured
  +24us from doubled stationary reloads, unmodeled in CoreSim).
  Masks ship as bf16 {0,1} with a 5-chunk-lead DMA cursor; head tensors
  prefetch ~1.5 slices early so their 7us of DMA never starves the mask
  stream. The 1/(1-p)=2 dropout rescale is folded into the 0.5-valued ones
  weights: out = oacc / (0.5 * sum_k exp).
"""

import numpy as np
from contextlib import ExitStack

import concourse.bass as bass
import concourse.bacc as bacc
import concourse.tile as tile
import concourse.mybir as mybir
from concourse.bass_utils import run_bass_kernel_spmd

N_CORES = 8
B, S, D = 64, 2048, 64
HPC = B // N_CORES  # heads per core
KP = 128            # k-chunk size (PSUM partition dim)
QL = 1024           # q-slice width (one [128,1024] PSUM score tile = 2 banks)
NQ = 512            # matmul moving free-dim tile (one fp32 PSUM bank)
DROP_P = 0.5
N_KC = S // KP      # 16 k-chunks
MK_LEAD = 5         # mask DMA prefetch lead, in chunks

# Chunks whose denominator ones-matmul runs directly on PE (rest are summed
# in bf16 on DVE, two accumulators, folded by PE at end of slice).
PE_CHUNKS = tuple(range(5))


def build_program(
    n_heads=HPC,
    seq=S,
    d=D,
    scale=1.0,
    reps=1,
    pe_chunks=PE_CHUNKS,
    mk_lead=MK_LEAD,
):
    f32 = mybir.dt.float32
    bf16 = mybir.dt.bfloat16
    # float32r: same fp32 bytes, PE streams 1 col/cycle (vs 4 for fp32) at
    # ~tf32 precision (HW-probed maxabs 5.8e-3 on N(0,64) scores).
    fmm = mybir.dt.float32r
    n_kc = seq // KP
    n_qh = seq // QL
    n_j = QL // NQ
    pe_set = set(c for c in pe_chunks if c < n_kc)
    dve_accs = [c for c in range(n_kc) if c not in pe_set]
    acc_of = {}
    for i, c in enumerate(dve_accs):
        acc_of[c] = 0 if i < (len(dve_accs) + 1) // 2 else 1

    nc = bacc.Bacc("TRN2", target_bir_lowering=False, debug=False)
    qt_d = nc.dram_tensor("qt", [n_heads, d, seq], fmm, kind="ExternalInput").ap()
    kt_d = nc.dram_tensor("kt", [n_heads, d, seq], fmm, kind="ExternalInput").ap()
    vp_d = nc.dram_tensor("vp", [n_heads, KP, n_kc * d], bf16, kind="ExternalInput").ap()
    mt_d = nc.dram_tensor("mt", [n_heads, seq, seq], bf16, kind="ExternalInput").ap()
    ot_d = nc.dram_tensor("ot", [n_heads, d, seq], f32, kind="ExternalOutput").ap()

    # Software-pipelined emission over a flat list of (head, q-slice) blocks:
    # per chunk c the program order is [dma mask(cursor)] [exp(c)] [QK(next)]
    # [mask-mult(c)] [PV(c)] [denom(c)], so each engine's in-order stream
    # never waits on the current chunk's cross-engine chain.
    blocks = [(h, qh) for h in range(n_heads) for qh in range(n_qh)] * reps

    with tile.TileContext(nc) as tc:
        with ExitStack() as ctx:
            const = ctx.enter_context(tc.tile_pool(name="const", bufs=1))
            qkv = ctx.enter_context(tc.tile_pool(name="qkv", bufs=2))
            mpool = ctx.enter_context(tc.tile_pool(name="mask", bufs=12))
            ppool = ctx.enter_context(tc.tile_pool(name="p", bufs=8))
            dpool = ctx.enter_context(tc.tile_pool(name="pd", bufs=5))
            apool = ctx.enter_context(tc.tile_pool(name="acc", bufs=4))
            opool = ctx.enter_context(tc.tile_pool(name="o", bufs=3))
            # PSUM budget (8 banks): st 2x2 + oacc 2 + oden 2.
            pst = ctx.enter_context(
                tc.tile_pool(name="pst", bufs=2, space=bass.MemorySpace.PSUM)
            )
            pacc = ctx.enter_context(
                tc.tile_pool(name="pacc", bufs=1, space=bass.MemorySpace.PSUM)
            )
            pden = ctx.enter_context(
                tc.tile_pool(name="pden", bufs=1, space=bass.MemorySpace.PSUM)
            )

            # d identical 0.5-columns: the denominator matmul then emits
            # 0.5*sum_k already replicated across the d output partitions,
            # and the 0.5 folds the dropout 1/(1-p)=2 rescale into the
            # final reciprocal.
            ones = const.tile([KP, d], bf16)
            nc.vector.memset(ones[:], 0.5)

            head_tiles: dict = {}

            def load_head(h):
                qt_sb = qkv.tile([d, seq], fmm, tag="qt")
                nc.sync.dma_start(qt_sb[:], qt_d[h])
                kt_sb = qkv.tile([d, seq], fmm, tag="kt")
                nc.sync.dma_start(kt_sb[:], kt_d[h])
                v_sb = qkv.tile([KP, n_kc * d], bf16, tag="v")
                nc.sync.dma_start(v_sb[:], vp_d[h])
                head_tiles[h] = (qt_sb, kt_sb, v_sb)

            mk_tiles: dict = {}
            st_tiles: dict = {}

            def dma_mk(b, c):
                h, qh = blocks[b]
                q0 = qh * QL
                t = mpool.tile([KP, QL], bf16, tag="mk")
                nc.sync.dma_start(t[:], mt_d[h, c * KP : (c + 1) * KP, q0 : q0 + QL])
                mk_tiles[(b, c)] = t

            def qk(b, c):
                h, qh = blocks[b]
                q0 = qh * QL
                qt_sb, kt_sb, _ = head_tiles[h]
                t = pst.tile([KP, QL], f32, tag="st")
                for j in range(n_j):
                    nc.tensor.matmul(
                        t[:, j * NQ : (j + 1) * NQ],
                        kt_sb[:, c * KP : (c + 1) * KP],
                        qt_sb[:, q0 + j * NQ : q0 + (j + 1) * NQ],
                        start=True,
                        stop=True,
                    )
                st_tiles[(b, c)] = t

            mk_sched = [(bb, cc) for bb in range(len(blocks)) for cc in range(n_kc)]
            mk_cursor = [0]

            def advance_mk(n):
                for _ in range(n):
                    if mk_cursor[0] < len(mk_sched):
                        dma_mk(*mk_sched[mk_cursor[0]])
                        mk_cursor[0] += 1

            load_head(0)
            advance_mk(mk_lead)
            qk(0, 0)

            pe_sorted = sorted(pe_set)
            half = (len(dve_accs) + 1) // 2
            n_dsrc = (
                len(pe_sorted) + (1 if half else 0) + (1 if len(dve_accs) - half else 0)
            )

            pending = [None, None]  # deferred out-stage compute / dma

            for b, (h, qh) in enumerate(blocks):
                _, _, v_sb = head_tiles[h]
                oacc = pacc.tile([d, QL], f32, tag="oacc")
                oden = pden.tile([d, QL], f32, tag="oden")
                accs = [None, None]
                pend = [None, None]  # first p0 of an accumulator pair
                dsrc = [0]

                def oden_fold(src, oden=oden, dsrc=dsrc):
                    for j in range(n_j):
                        nc.tensor.matmul(
                            oden[:, j * NQ : (j + 1) * NQ],
                            ones,
                            src[:, j * NQ : (j + 1) * NQ],
                            start=dsrc[0] == 0,
                            stop=dsrc[0] == n_dsrc - 1,
                        )
                    dsrc[0] += 1

                for c in range(n_kc):
                    # prefetch the next head's tensors halfway through this
                    # head's FIRST slice (~17us lead over first use)
                    if (
                        c == n_kc // 2
                        and qh == 0
                        and b + 2 < len(blocks)
                        and blocks[b + 2][0] != h
                    ):
                        load_head(blocks[b + 2][0])
                    advance_mk(1)

                    st = st_tiles.pop((b, c))
                    p0 = ppool.tile([KP, QL], bf16, tag="p0")
                    nc.scalar.activation(
                        p0[:], st[:], mybir.ActivationFunctionType.Exp, scale=scale
                    )
                    nxt = (b, c + 1) if c + 1 < n_kc else (b + 1, 0)
                    if nxt[0] < len(blocks):
                        qk(*nxt)
                    mk = mk_tiles.pop((b, c))
                    pd = dpool.tile([KP, QL], bf16, tag="pd")
                    nc.vector.tensor_tensor(pd[:], mk[:], p0[:], mybir.AluOpType.mult)
                    if c == 0 and pending[0] is not None:
                        pending[1] = pending[0]()
                        pending[0] = None
                    elif c == 1 and pending[1] is not None:
                        pending[1]()
                        pending[1] = None
                    first, last = c == 0, c == n_kc - 1
                    for j in range(n_j):
                        nc.tensor.matmul(
                            oacc[:, j * NQ : (j + 1) * NQ],
                            v_sb[:, c * d : (c + 1) * d],
                            pd[:, j * NQ : (j + 1) * NQ],
                            start=first,
                            stop=last,
                        )
                    # denominator contribution of this chunk
                    if c in pe_set:
                        oden_fold(p0)
                    else:
                        ai = acc_of[c]
                        if accs[ai] is None and pend[ai] is None:
                            pend[ai] = p0
                        elif accs[ai] is None:
                            t = apool.tile([KP, QL], bf16, tag="acc")
                            nc.vector.tensor_tensor(
                                t[:], pend[ai][:], p0[:], mybir.AluOpType.add
                            )
                            accs[ai] = t
                            pend[ai] = None
                        else:
                            nc.vector.tensor_tensor(
                                accs[ai][:], accs[ai][:], p0[:], mybir.AluOpType.add
                            )

                # fold the two bf16 accumulators into the PSUM denominator
                for acc in accs:
                    if acc is not None:
                        oden_fold(acc)
                for pp in pend:
                    if pp is not None:
                        oden_fold(pp)

                # out = oacc * (1 / (0.5 * sum_k exp)); the compute defers to
                # the next slice's first mask-mult and the store one chunk
                # further, keeping both off the in-order critical paths.
                def make_out(h=h, qh=qh, oacc=oacc, oden=oden):
                    def emit():
                        q0 = qh * QL
                        rb = opool.tile([d, QL], f32, tag="rb")
                        nc.vector.reciprocal_approx_fast(rb[:], oden[:])
                        out_sb = opool.tile([d, QL], f32, tag="out")
                        nc.vector.tensor_tensor(
                            out_sb[:], oacc[:], rb[:], mybir.AluOpType.mult
                        )

                        def emit_dma():
                            nc.sync.dma_start(ot_d[h, :, q0 : q0 + QL], out_sb[:])

                        return emit_dma

                    return emit

                pending[0] = make_out()
            if pending[0] is not None:
                pending[1] = pending[0]()
            if pending[1] is not None:
                pending[1]()

    nc.compile()
    return nc


_CACHE: dict = {}


def _get_program(scale: float):
    key = float(scale)
    if key not in _CACHE:
        _CACHE[key] = build_program(scale=key)
    return _CACHE[key]


def make_in_maps(query, key, value, dropout_mask, **_ignored):
    """Shard + relayout the full inputs into the 8 per-core input maps."""
    import ml_dtypes

    query = np.asarray(query, dtype=np.float32)
    key = np.asarray(key, dtype=np.float32)
    value = np.asarray(value, dtype=np.float32)
    dropout_mask = np.asarray(dropout_mask, dtype=np.float32)
    in_maps = []
    for cid in range(N_CORES):
        sl = slice(cid * HPC, (cid + 1) * HPC)
        qt = np.ascontiguousarray(query[sl].transpose(0, 2, 1))
        kt = np.ascontiguousarray(key[sl].transpose(0, 2, 1))
        vp = np.ascontiguousarray(
            value[sl].reshape(HPC, S // KP, KP, D).transpose(0, 2, 1, 3)
        ).reshape(HPC, KP, (S // KP) * D).astype(ml_dtypes.bfloat16)
        mt = (dropout_mask[sl].transpose(0, 2, 1) >= DROP_P).astype(
            ml_dtypes.bfloat16
        )  # [h, k, q] keep-mask
        in_maps.append({"qt": qt, "kt": kt, "vp": vp, "mt": mt})
    return in_maps


def run(query, key, value, scale_factor, dropout_mask, trace=False, **trace_kwargs):
    scale = float(np.asarray(scale_factor).reshape(()))
    nc = _get_program(scale)
    in_maps = make_in_maps(query, key, value, dropout_mask)
    res = run_bass_kernel_spmd(
        nc, in_maps, core_ids=list(range(N_CORES)), trace=trace, **trace_kwargs
    )
    outs = [res.results[c]["ot"].transpose(0, 2, 1) for c in range(N_CORES)]
    full = np.ascontiguousarray(np.concatenate(outs, axis=0), dtype=np.float32)
    return full, res


def kernel(query, key, value, scale_factor, dropout_mask):
    out, _ = run(query, key, value, scale_factor, dropout_mask, trace=False)
    return out


# revision 32
# speedup vs baseline: 1.2478x; 1.0130x over previous
"""Fused multi-head attention with dropout for Trainium2 (Bass/Tile), 8-core SPMD.

Problem: out = dropout(softmax(Q @ K^T * scale)) @ V
  Q/K/V: [64, 2048, 64] fp32, dropout_mask: [64, 2048, 2048] fp32, p = 0.5.

Sharding: the 64 batch*heads are split across 8 NeuronCores (8 heads/core),
no cross-device communication.

Per-head device algorithm (head-local, S = 2048, D = 64):
  Scores are computed TRANSPOSED, S^T[k, q] = K @ Q^T, so softmax rows (over
  k) land on the partition axis and the PV product needs no on-chip transpose:
  O^T[d, q] = sum_k V[k, d] * P[k, q] accumulates in PSUM.

  Engine balance (the baseline was PE-bound at ~327us/core because the
  softmax denominator sum_k exp(s) was a ones-matmul per k-chunk - 1/3 of
  all PE cycles - with Vector near-saturated and GpSimd idle):
   - Act: exp only ([128,1024] fp32 PSUM tiles -> bf16 SBUF), nothing else;
          at 1.2G rows/s + ~185ns/instr this is the ~266us pipeline pacer.
   - PE:  QK (fp32r) + PV (bf16) + denominator ones-matmuls for PE_CHUNKS
          + accumulator folds at end of slice (end placement matters: a
          fold mid-slice stalls the in-order PE queue on the DVE chain).
   - DVE: dropout mask-mult as all-bf16 tensor_tensor (the 2x_1p DVE mode
          needs every operand 2-byte; HW-measured 682ns/[128,1024] tile vs
          3.7us for any u8-mixed op), two bf16 denominator chunk-sum
          accumulators, reciprocal + final output multiply, both deferred
          into the next slice so they stay off the critical path.
   - GpSimd: nothing (a dependent gpsimd op costs ~10us pipeline latency on
          HW, and even its software-DGE DMAs measure ~20us slower).
  Chunk-major [128,1024] score tiles keep one stationary load per two
  matmuls (a 512-wide variant that halved the accumulator banks measured
  +24us from doubled stationary reloads, unmodeled in CoreSim).
  Masks ship as bf16 {0,1} with a 5-chunk-lead DMA cursor; head tensors
  prefetch ~1.5 slices early so their 7us of DMA never starves the mask
  stream. The 1/(1-p)=2 dropout rescale is folded into the 0.5-valued ones
  weights: out = oacc / (0.5 * sum_k exp).
"""

import numpy as np
from contextlib import ExitStack

import concourse.bass as bass
import concourse.bacc as bacc
import concourse.tile as tile
import concourse.mybir as mybir
from concourse.bass_utils import run_bass_kernel_spmd

N_CORES = 8
B, S, D = 64, 2048, 64
HPC = B // N_CORES  # heads per core
KP = 128            # k-chunk size (PSUM partition dim)
QL = 1024           # q-slice width (one [128,1024] PSUM score tile = 2 banks)
NQ = 512            # matmul moving free-dim tile (one fp32 PSUM bank)
DROP_P = 0.5
N_KC = S // KP      # 16 k-chunks
MK_LEAD = 5         # mask DMA prefetch lead, in chunks

# Chunks whose denominator ones-matmul runs directly on PE (rest are summed
# in bf16 on DVE, two accumulators, folded by PE at end of slice). Chunk 15
# is PE-direct so the end-of-slice chain exp(15)->fold->recip never waits on
# a DVE add: the accumulators complete at c=14, ahead of the tail (HW: 313.8us
# vs 322.7us with chunks 0..4).
PE_CHUNKS = (0, 1, 2, 3, 15)


def build_program(
    n_heads=HPC,
    seq=S,
    d=D,
    scale=1.0,
    reps=1,
    pe_chunks=PE_CHUNKS,
    mk_lead=MK_LEAD,
):
    f32 = mybir.dt.float32
    bf16 = mybir.dt.bfloat16
    # float32r: same fp32 bytes, PE streams 1 col/cycle (vs 4 for fp32) at
    # ~tf32 precision (HW-probed maxabs 5.8e-3 on N(0,64) scores).
    fmm = mybir.dt.float32r
    n_kc = seq // KP
    n_qh = seq // QL
    n_j = QL // NQ
    pe_set = set(c for c in pe_chunks if c < n_kc)
    dve_accs = [c for c in range(n_kc) if c not in pe_set]
    acc_of = {}
    for i, c in enumerate(dve_accs):
        acc_of[c] = 0 if i < (len(dve_accs) + 1) // 2 else 1

    nc = bacc.Bacc("TRN2", target_bir_lowering=False, debug=False)
    qt_d = nc.dram_tensor("qt", [n_heads, d, seq], fmm, kind="ExternalInput").ap()
    kt_d = nc.dram_tensor("kt", [n_heads, d, seq], fmm, kind="ExternalInput").ap()
    vp_d = nc.dram_tensor("vp", [n_heads, KP, n_kc * d], bf16, kind="ExternalInput").ap()
    mt_d = nc.dram_tensor("mt", [n_heads, seq, seq], bf16, kind="ExternalInput").ap()
    ot_d = nc.dram_tensor("ot", [n_heads, d, seq], f32, kind="ExternalOutput").ap()

    # Software-pipelined emission over a flat list of (head, q-slice) blocks:
    # per chunk c the program order is [dma mask(cursor)] [exp(c)] [QK(next)]
    # [mask-mult(c)] [PV(c)] [denom(c)], so each engine's in-order stream
    # never waits on the current chunk's cross-engine chain.
    blocks = [(h, qh) for h in range(n_heads) for qh in range(n_qh)] * reps

    with tile.TileContext(nc) as tc:
        with ExitStack() as ctx:
            const = ctx.enter_context(tc.tile_pool(name="const", bufs=1))
            qkv = ctx.enter_context(tc.tile_pool(name="qkv", bufs=2))
            mpool = ctx.enter_context(tc.tile_pool(name="mask", bufs=12))
            ppool = ctx.enter_context(tc.tile_pool(name="p", bufs=8))
            dpool = ctx.enter_context(tc.tile_pool(name="pd", bufs=5))
            apool = ctx.enter_context(tc.tile_pool(name="acc", bufs=4))
            opool = ctx.enter_context(tc.tile_pool(name="o", bufs=3))
            # PSUM budget (8 banks): st 2x2 + oacc 2 + oden 2.
            pst = ctx.enter_context(
                tc.tile_pool(name="pst", bufs=2, space=bass.MemorySpace.PSUM)
            )
            pacc = ctx.enter_context(
                tc.tile_pool(name="pacc", bufs=1, space=bass.MemorySpace.PSUM)
            )
            pden = ctx.enter_context(
                tc.tile_pool(name="pden", bufs=1, space=bass.MemorySpace.PSUM)
            )

            # d identical 0.5-columns: the denominator matmul then emits
            # 0.5*sum_k already replicated across the d output partitions,
            # and the 0.5 folds the dropout 1/(1-p)=2 rescale into the
            # final reciprocal.
            ones = const.tile([KP, d], bf16)
            nc.vector.memset(ones[:], 0.5)

            head_tiles: dict = {}

            def load_head(h):
                qt_sb = qkv.tile([d, seq], fmm, tag="qt")
                nc.sync.dma_start(qt_sb[:], qt_d[h])
                kt_sb = qkv.tile([d, seq], fmm, tag="kt")
                nc.sync.dma_start(kt_sb[:], kt_d[h])
                v_sb = qkv.tile([KP, n_kc * d], bf16, tag="v")
                nc.sync.dma_start(v_sb[:], vp_d[h])
                head_tiles[h] = (qt_sb, kt_sb, v_sb)

            mk_tiles: dict = {}
            st_tiles: dict = {}

            def dma_mk(b, c):
                h, qh = blocks[b]
                q0 = qh * QL
                t = mpool.tile([KP, QL], bf16, tag="mk")
                nc.sync.dma_start(t[:], mt_d[h, c * KP : (c + 1) * KP, q0 : q0 + QL])
                mk_tiles[(b, c)] = t

            def qk(b, c):
                h, qh = blocks[b]
                q0 = qh * QL
                qt_sb, kt_sb, _ = head_tiles[h]
                t = pst.tile([KP, QL], f32, tag="st")
                for j in range(n_j):
                    nc.tensor.matmul(
                        t[:, j * NQ : (j + 1) * NQ],
                        kt_sb[:, c * KP : (c + 1) * KP],
                        qt_sb[:, q0 + j * NQ : q0 + (j + 1) * NQ],
                        start=True,
                        stop=True,
                    )
                st_tiles[(b, c)] = t

            mk_sched = [(bb, cc) for bb in range(len(blocks)) for cc in range(n_kc)]
            mk_cursor = [0]

            def advance_mk(n):
                for _ in range(n):
                    if mk_cursor[0] < len(mk_sched):
                        dma_mk(*mk_sched[mk_cursor[0]])
                        mk_cursor[0] += 1

            load_head(0)
            advance_mk(mk_lead)
            qk(0, 0)

            pe_sorted = sorted(pe_set)
            half = (len(dve_accs) + 1) // 2
            n_dsrc = (
                len(pe_sorted) + (1 if half else 0) + (1 if len(dve_accs) - half else 0)
            )

            pending = [None, None]  # deferred out-stage compute / dma

            for b, (h, qh) in enumerate(blocks):
                _, _, v_sb = head_tiles[h]
                oacc = pacc.tile([d, QL], f32, tag="oacc")
                oden = pden.tile([d, QL], f32, tag="oden")
                accs = [None, None]
                pend = [None, None]  # first p0 of an accumulator pair
                dsrc = [0]

                def oden_fold(src, oden=oden, dsrc=dsrc):
                    for j in range(n_j):
                        nc.tensor.matmul(
                            oden[:, j * NQ : (j + 1) * NQ],
                            ones,
                            src[:, j * NQ : (j + 1) * NQ],
                            start=dsrc[0] == 0,
                            stop=dsrc[0] == n_dsrc - 1,
                        )
                    dsrc[0] += 1

                for c in range(n_kc):
                    # prefetch the next head's tensors halfway through this
                    # head's FIRST slice (~17us lead over first use)
                    if (
                        c == n_kc // 2
                        and qh == 0
                        and b + 2 < len(blocks)
                        and blocks[b + 2][0] != h
                    ):
                        load_head(blocks[b + 2][0])
                    advance_mk(1)

                    st = st_tiles.pop((b, c))
                    p0 = ppool.tile([KP, QL], bf16, tag="p0")
                    nc.scalar.activation(
                        p0[:], st[:], mybir.ActivationFunctionType.Exp, scale=scale
                    )
                    nxt = (b, c + 1) if c + 1 < n_kc else (b + 1, 0)
                    if nxt[0] < len(blocks):
                        qk(*nxt)
                    mk = mk_tiles.pop((b, c))
                    pd = dpool.tile([KP, QL], bf16, tag="pd")
                    nc.vector.tensor_tensor(pd[:], mk[:], p0[:], mybir.AluOpType.mult)
                    if c == 0 and pending[0] is not None:
                        pending[1] = pending[0]()
                        pending[0] = None
                    elif c == 1 and pending[1] is not None:
                        pending[1]()
                        pending[1] = None
                    first, last = c == 0, c == n_kc - 1

                    def emit_denom(p0=p0, c=c):
                        if c in pe_set:
                            oden_fold(p0)
                        else:
                            ai = acc_of[c]
                            if accs[ai] is None and pend[ai] is None:
                                pend[ai] = p0
                            elif accs[ai] is None:
                                t = apool.tile([KP, QL], bf16, tag="acc")
                                nc.vector.tensor_tensor(
                                    t[:], pend[ai][:], p0[:], mybir.AluOpType.add
                                )
                                accs[ai] = t
                                pend[ai] = None
                            else:
                                nc.vector.tensor_tensor(
                                    accs[ai][:], accs[ai][:], p0[:],
                                    mybir.AluOpType.add,
                                )

                    def emit_folds():
                        for acc in accs:
                            if acc is not None:
                                oden_fold(acc)
                        for pp in pend:
                            if pp is not None:
                                oden_fold(pp)

                    # at the last chunk, emit denominator + folds BEFORE the
                    # PV matmuls: they depend only on exp(15)/completed accs,
                    # while PV(15) waits on the DVE mask-mult - this keeps
                    # the reciprocal's inputs from queueing behind a DVE
                    # wait in the in-order PE stream.
                    if last:
                        emit_denom()
                        emit_folds()
                    for j in range(n_j):
                        nc.tensor.matmul(
                            oacc[:, j * NQ : (j + 1) * NQ],
                            v_sb[:, c * d : (c + 1) * d],
                            pd[:, j * NQ : (j + 1) * NQ],
                            start=first,
                            stop=last,
                        )
                    if not last:
                        emit_denom()

                # out = oacc * (1 / (0.5 * sum_k exp)); the compute defers to
                # the next slice's first mask-mult and the store one chunk
                # further, keeping both off the in-order critical paths.
                def make_out(h=h, qh=qh, oacc=oacc, oden=oden):
                    def emit():
                        q0 = qh * QL
                        rb = opool.tile([d, QL], f32, tag="rb")
                        nc.vector.reciprocal_approx_fast(rb[:], oden[:])
                        out_sb = opool.tile([d, QL], f32, tag="out")
                        nc.vector.tensor_tensor(
                            out_sb[:], oacc[:], rb[:], mybir.AluOpType.mult
                        )

                        def emit_dma():
                            nc.sync.dma_start(ot_d[h, :, q0 : q0 + QL], out_sb[:])

                        return emit_dma

                    return emit

                pending[0] = make_out()
            if pending[0] is not None:
                pending[1] = pending[0]()
            if pending[1] is not None:
                pending[1]()

    nc.compile()
    return nc


_CACHE: dict = {}


def _get_program(scale: float):
    key = float(scale)
    if key not in _CACHE:
        _CACHE[key] = build_program(scale=key)
    return _CACHE[key]


def make_in_maps(query, key, value, dropout_mask, **_ignored):
    """Shard + relayout the full inputs into the 8 per-core input maps."""
    import ml_dtypes

    query = np.asarray(query, dtype=np.float32)
    key = np.asarray(key, dtype=np.float32)
    value = np.asarray(value, dtype=np.float32)
    dropout_mask = np.asarray(dropout_mask, dtype=np.float32)
    in_maps = []
    for cid in range(N_CORES):
        sl = slice(cid * HPC, (cid + 1) * HPC)
        qt = np.ascontiguousarray(query[sl].transpose(0, 2, 1))
        kt = np.ascontiguousarray(key[sl].transpose(0, 2, 1))
        vp = np.ascontiguousarray(
            value[sl].reshape(HPC, S // KP, KP, D).transpose(0, 2, 1, 3)
        ).reshape(HPC, KP, (S // KP) * D).astype(ml_dtypes.bfloat16)
        mt = (dropout_mask[sl].transpose(0, 2, 1) >= DROP_P).astype(
            ml_dtypes.bfloat16
        )  # [h, k, q] keep-mask
        in_maps.append({"qt": qt, "kt": kt, "vp": vp, "mt": mt})
    return in_maps


def run(query, key, value, scale_factor, dropout_mask, trace=False, **trace_kwargs):
    scale = float(np.asarray(scale_factor).reshape(()))
    nc = _get_program(scale)
    in_maps = make_in_maps(query, key, value, dropout_mask)
    res = run_bass_kernel_spmd(
        nc, in_maps, core_ids=list(range(N_CORES)), trace=trace, **trace_kwargs
    )
    outs = [res.results[c]["ot"].transpose(0, 2, 1) for c in range(N_CORES)]
    full = np.ascontiguousarray(np.concatenate(outs, axis=0), dtype=np.float32)
    return full, res


def kernel(query, key, value, scale_factor, dropout_mask):
    out, _ = run(query, key, value, scale_factor, dropout_mask, trace=False)
    return out
